# revision 1
# baseline (speedup 1.0000x reference)
"""AdaptiveFeaturePropagation Trainium2 kernel (8 NeuronCores, SPMD).

Sharding: 8 cores = (batch 4) x (H halves 2); halos replicated host-side, no
collectives. Per core (shard rows [s, s+32)):
  conv1 (3x3, 1024->256, applied to cur & key): fp32r matmuls, contraction
    over Cin in 128-chunks x 9 taps accumulated in PSUM.
  conv2 (3x3, 512->256) and conv3 (1x1, 256->81): bf16.
  conv3 is computed "swapped" (x3 pixel-block as the stationary operand) so
  kernel logits land pixel-major [128 pix, 81]; softmax = Relu+Exp on ACT with
  accum_out giving the denominator; normalization folded into the SVC drain.
  Spatially-variant 9x9 conv: banded-matrix matmul
    out[pix, c] = sum_g band_g[s,j].T @ highT[g][s, c]
  with the band built by a DMA scatter into a DRAM image at addr 129*p+128*k
  (zero margins), loaded back contiguously, and a static 0/1 mask zeroing
  aliased tap slots.
Output is written pixel-major [2048, 1024] per core; host transposes.
"""

import sys

sys.path.insert(0, "/opt/trn_rl_repo")

import numpy as np
import ml_dtypes

import concourse.bacc as bacc
import concourse.mybir as mybir
from concourse.bass_types import AP
from concourse.tile import TileContext
from concourse.bass_utils import run_bass_kernel_spmd

F32 = mybir.dt.float32
F32R = mybir.dt.float32r
BF16 = mybir.dt.bfloat16
BF = ml_dtypes.bfloat16
AF = mybir.ActivationFunctionType

# ---------------- configuration ----------------


class Cfg:
    B = 4
    HALVES = 2
    H = 64
    W = 64
    C_IN = 1024  # conv1 input channels
    CO1 = 256  # conv1 output channels (per branch)
    C2 = 512  # conv2 input channels
    CO2 = 256  # conv2 output channels
    K81 = 81
    CH = 1024  # high-feature channels

    OUT_ROWS = 32  # output rows per shard
    # conv1 output rows = OUT_ROWS + 2 (halo +-1), input rows = OUT_ROWS + 4
    # conv1 row blocks: (o0, out_rows); all N = out_rows*64 >= 256 for fp32r
    C1_BLOCKS = ((0, 7), (7, 7), (14, 7), (21, 7), (28, 6))

    @property
    def X2_ROWS(self):
        return self.OUT_ROWS + 2

    @property
    def IN_ROWS(self):
        return self.OUT_ROWS + 4

    @property
    def HT_ROWS(self):
        return self.OUT_ROWS + 8

    @property
    def PAIRS(self):
        return self.OUT_ROWS // 2

    @property
    def WP(self):
        return self.W + 2

    @property
    def WH(self):
        return self.W + 8

    # band image geometry: addr = 129*p + 128*k, p in [0,128), k in [0,81)
    XPW = 129  # p stride
    XKW = 128  # k stride

    @property
    def XR(self):  # r offset: 129*64
        return self.XPW * self.W

    @property
    def XSZ(self):  # per-pair region size (max read addr + pad)
        return 26752


CFG = Cfg()

# ---------------- graph builder ----------------


def build_graph(cfg):
    nc = bacc.Bacc(None, target_bir_lowering=False)
    W = cfg.W
    n_cin_ch = cfg.C_IN // 128
    n_c2_ch = cfg.C2 // 128
    n_co1_h = cfg.CO1 // 128
    n_co2_h = cfg.CO2 // 128
    n_cc = cfg.CH // 512  # SVC c-chunks
    PAIRS = cfg.PAIRS

    cur_e = nc.declare_dram_parameter(
        "cur", [cfg.C_IN, cfg.IN_ROWS, cfg.WP], F32R, isOutput=False
    )
    key_e = nc.declare_dram_parameter(
        "key", [cfg.C_IN, cfg.IN_ROWS, cfg.WP], F32R, isOutput=False
    )
    high_e = nc.declare_dram_parameter(
        "highT", [cfg.HT_ROWS, cfg.WH, cfg.CH], BF16, isOutput=False
    )
    w1_e = nc.declare_dram_parameter(
        "w1", [128, 9, n_cin_ch, cfg.CO1], F32R, isOutput=False
    )
    w2_e = nc.declare_dram_parameter(
        "w2", [128, 9, n_c2_ch, cfg.CO2], BF16, isOutput=False
    )
    w3_e = nc.declare_dram_parameter(
        "w3", [128, cfg.CO2 // 128, cfg.K81], BF16, isOutput=False
    )
    b1_e = nc.declare_dram_parameter("b1", [128, n_co1_h], F32, isOutput=False)
    b2_e = nc.declare_dram_parameter("b2", [128, n_co2_h], F32, isOutput=False)
    b3_e = nc.declare_dram_parameter("b3", [128, cfg.K81], F32, isOutput=False)
    hmask_e = nc.declare_dram_parameter("hmask", [128, 2], F32, isOutput=False)
    bmask_e = nc.declare_dram_parameter(
        "bmask", [cfg.WH, 10 * 2 * W], BF16, isOutput=False
    )
    out_e = nc.declare_dram_parameter(
        "out", [cfg.OUT_ROWS * W, cfg.CH], F32, isOutput=True
    )

    ximg = nc.dram_tensor("ximg", [PAIRS * cfg.XSZ], BF16)

    with TileContext(nc) as tc:
        with (
            tc.tile_pool(name="const", bufs=1) as cpool,
            tc.tile_pool(name="feat", bufs=1) as fpool,
            tc.tile_pool(name="c1in", bufs=3) as inpool,
            tc.tile_pool(name="ht", bufs=12) as htpool,
            tc.tile_pool(name="band", bufs=8) as bandpool,
            tc.tile_pool(name="small", bufs=8) as spool,
            tc.tile_pool(name="ob", bufs=3) as obpool,
            tc.tile_pool(name="ps", bufs=8, space="PSUM") as pspool,
        ):
            # ---- persistent constants ----
            w1sb = cpool.tile([128, 9 * n_cin_ch * cfg.CO1], F32R)
            nc.scalar.dma_start(out=w1sb[:], in_=w1_e[:, :, :, :])
            b1sb = cpool.tile([128, n_co1_h], F32)
            nc.scalar.dma_start(out=b1sb[:], in_=b1_e[:, :])
            hmsb = cpool.tile([128, 2], F32)
            nc.scalar.dma_start(out=hmsb[:], in_=hmask_e[:, :])
            w2sb = cpool.tile([128, 9 * n_c2_ch * cfg.CO2], BF16)
            w3sb = cpool.tile([128, (cfg.CO2 // 128) * cfg.K81], BF16)
            b2sb = cpool.tile([128, n_co2_h], F32)
            b3sb = cpool.tile([128, cfg.K81], F32)
            bmsb = cpool.tile([cfg.WH, 10 * 2 * W], BF16)
            zt = cpool.tile([128, cfg.XSZ // 128], BF16)

            def emit_deferred_consts():
                nc.scalar.dma_start(out=w2sb[:], in_=w2_e[:, :, :, :])
                nc.scalar.dma_start(out=w3sb[:], in_=w3_e[:, :, :])
                nc.scalar.dma_start(out=b2sb[:], in_=b2_e[:, :])
                nc.scalar.dma_start(out=b3sb[:], in_=b3_e[:, :])
                nc.scalar.dma_start(out=bmsb[:], in_=bmask_e[:, :])
                nc.vector.memset(zt[:], 0.0)
                for t in range(PAIRS):
                    dz = AP(
                        ximg,
                        t * cfg.XSZ,
                        [[cfg.XSZ // 128, 128], [1, cfg.XSZ // 128]],
                    )
                    nc.scalar.dma_start(out=dz, in_=zt[:])

            # x2 (conv1 out, conv2 in), bf16, padded cols; x3 (conv2 out)
            x2c = []
            for i in range(2 * n_co1_h):
                t_ = fpool.tile([128, cfg.X2_ROWS * cfg.WP], BF16, tag=f"x2_{i}")
                nc.vector.memset(t_[:], 0.0)
                x2c.append(t_)
            x3c = []
            for i in range(n_co2_h):
                t_ = fpool.tile([128, cfg.OUT_ROWS * W], BF16, tag=f"x3_{i}")
                x3c.append(t_)

            # highT ring
            ht = {}

            def need_ht(g):
                if g not in ht:
                    h_ = htpool.tile([cfg.WH, cfg.CH], BF16, tag="ht")
                    nc.scalar.dma_start(out=h_[:], in_=high_e[g, :, :])
                    ht[g] = h_
                return ht[g]


            # ---- conv1 (cur, key) -> x2 ----
            def emit_c1_block(bi):
                o0, nout = cfg.C1_BLOCKS[bi]
                nin = nout + 2
                for ii, inp_e in enumerate((cur_e, key_e)):
                    pss = [
                        pspool.tile([128, nout * W], F32, tag="ps", name=f"ps1_{o0}_{ii}_{h_}")
                        for h_ in range(n_co1_h)
                    ]
                    for ch in range(n_cin_ch):
                        it = inpool.tile([128, nin * cfg.WP], F32R, tag="c1in")
                        nc.scalar.dma_start(
                            out=it[:],
                            in_=inp_e[ch * 128 : (ch + 1) * 128, o0 : o0 + nin, :],
                        )
                        itv = it[:, :].rearrange("p (r w) -> p r w", w=cfg.WP)
                        for tap in range(9):
                            dy, dx = tap // 3, tap % 3
                            rhs = itv[:, dy : dy + nout, dx : dx + W]
                            for hf in range(n_co1_h):
                                lhsT = w1sb[
                                    :,
                                    (tap * n_cin_ch + ch) * cfg.CO1
                                    + 128 * hf : (tap * n_cin_ch + ch) * cfg.CO1
                                    + 128 * (hf + 1),
                                ]
                                nc.tensor.matmul(
                                    pss[hf][:, :],
                                    lhsT,
                                    rhs,
                                    start=(ch == 0 and tap == 0),
                                    stop=(ch == n_cin_ch - 1 and tap == 8),
                                )
                    for hf in range(n_co1_h):
                        dst = x2c[ii * n_co1_h + hf][:, :].rearrange(
                            "p (r w) -> p r w", w=cfg.WP
                        )[:, o0 : o0 + nout, 1 : 1 + W]
                        nc.scalar.activation(
                            dst,
                            pss[hf][:, :],
                            AF.Relu,
                            bias=b1sb[:, hf : hf + 1],
                        )

            # halo row masking (rows 0 and X2_ROWS-1 of x2)
            lr = cfg.X2_ROWS - 1
            def emit_mask_top():
                for i in range(2 * n_co1_h):
                    nc.vector.tensor_scalar_mul(
                        x2c[i][:, 0 : cfg.WP], x2c[i][:, 0 : cfg.WP], hmsb[:, 0:1]
                    )
            def emit_mask_bot():
                for i in range(2 * n_co1_h):
                    nc.vector.tensor_scalar_mul(
                        x2c[i][:, lr * cfg.WP : (lr + 1) * cfg.WP],
                        x2c[i][:, lr * cfg.WP : (lr + 1) * cfg.WP],
                        hmsb[:, 1:2],
                    )

            # ---- conv2 -> x3 ----
            def emit_c2_block(b):
                pss = [
                    pspool.tile([128, 4 * W], F32, tag="ps", name=f"ps2_{b}_{h_}")
                    for h_ in range(n_co2_h)
                ]
                for ch in range(n_c2_ch):
                    x2v = x2c[ch][:, :].rearrange("p (r w) -> p r w", w=cfg.WP)
                    for tap in range(9):
                        dy, dx = tap // 3, tap % 3
                        rhs = x2v[:, 4 * b + dy : 4 * b + dy + 4, dx : dx + W]
                        for hf in range(n_co2_h):
                            lhsT = w2sb[
                                :,
                                (tap * n_c2_ch + ch) * cfg.CO2
                                + 128 * hf : (tap * n_c2_ch + ch) * cfg.CO2
                                + 128 * (hf + 1),
                            ]
                            nc.tensor.matmul(
                                pss[hf][:, :],
                                lhsT,
                                rhs,
                                start=(ch == 0 and tap == 0),
                                stop=(ch == n_c2_ch - 1 and tap == 8),
                            )
                for hf in range(n_co2_h):
                    nc.scalar.activation(
                        x3c[hf][:, b * 4 * W : (b + 1) * 4 * W],
                        pss[hf][:, :],
                        AF.Relu,
                        bias=b2sb[:, hf : hf + 1],
                    )

            # ---- per row-pair: conv3 + softmax + band + SVC ----
            def emit_conv3(t):
                ps3 = pspool.tile([128, cfg.K81], F32, tag="ps")
                for ch in range(cfg.CO2 // 128):
                    nc.tensor.matmul(
                        ps3[:, :],
                        x3c[ch][:, t * 128 : (t + 1) * 128],
                        w3sb[:, ch * cfg.K81 : (ch + 1) * cfg.K81],
                        start=(ch == 0),
                        stop=(ch == cfg.CO2 // 128 - 1),
                    )
                t81 = spool.tile([128, cfg.K81], F32, tag="t81")
                nc.vector.tensor_add(t81[:], ps3[:, :], b3sb[:])
                nc.scalar.activation(t81[:], t81[:], AF.Relu)
                kt = spool.tile([128, cfg.K81], BF16, tag="kt")
                dsum = spool.tile([128, 1], F32, tag="dsum")
                nc.scalar.activation(kt[:], t81[:], AF.Exp, accum_out=dsum[:])
                rd = spool.tile([128, 1], F32, tag="rd")
                nc.vector.reciprocal(rd[:], dsum[:])
                # scatter into band image (SWDGE: separate queue from sync DMAs)
                for r in range(2):
                    dstap = AP(
                        ximg,
                        t * cfg.XSZ + cfg.XR * r,
                        [[cfg.XPW, W], [cfg.XKW, cfg.K81]],
                    )
                    nc.gpsimd.dma_start(out=dstap, in_=kt[W * r : W * (r + 1), :])
                # band load + mask
                band = bandpool.tile([cfg.WH, 10 * 2 * W], BF16, tag="band")
                for r in range(2):
                    srcap = AP(
                        ximg,
                        t * cfg.XSZ + (cfg.XR - 9 * cfg.XKW) * r,
                        [[cfg.XKW, cfg.WH], [cfg.XKW * 9, 10], [1, W]],
                    )
                    dstap = band[:, :].rearrange(
                        "s (g rr w) -> s g rr w", g=10, rr=2
                    )[:, :, r, :]
                    nc.sync.dma_start(out=dstap, in_=srcap)
                nc.vector.tensor_mul(band[:], band[:], bmsb[:])
                return band, rd

            def emit_svc(t, band, rd):
                for g in range(2 * t, 2 * t + 10):
                    need_ht(g)
                for cc in range(n_cc):
                    pv = pspool.tile([128, 512], F32, tag="ps")
                    for gi in range(10):
                        nc.tensor.matmul(
                            pv[:, :],
                            band[:, 128 * gi : 128 * (gi + 1)],
                            ht[2 * t + gi][:, 512 * cc : 512 * (cc + 1)],
                            start=(gi == 0),
                            stop=(gi == 9),
                        )
                    ob = obpool.tile([128, 512], F32, tag="ob")
                    nc.scalar.activation(
                        ob[:], pv[:, :], AF.Copy, scale=rd[:, 0:1]
                    )
                    nc.sync.dma_start(
                        out=out_e[t * 128 : (t + 1) * 128, 512 * cc : 512 * (cc + 1)],
                        in_=ob[:],
                    )

            # interleaved schedule: spread band scatters across conv compute
            chains = {}

            def emit_group(ts):
                for t in ts:
                    chains[t] = emit_conv3(t)

            def emit_svcs(ts):
                for t in ts:
                    emit_svc(t, *chains.pop(t))

            n_c2b = cfg.OUT_ROWS // 4
            if n_c2b == 8 and len(cfg.C1_BLOCKS) == 5:
                emit_c1_block(0)
                emit_mask_top()
                emit_deferred_consts()
                for g in range(10):
                    need_ht(g)
                emit_c1_block(1)
                emit_c2_block(0)
                emit_group([0, 1])
                emit_c2_block(1)
                emit_c2_block(2)
                emit_group([2, 3, 4, 5])
                emit_c1_block(2)
                emit_c2_block(3)
                emit_group([6, 7])
                emit_svcs([0, 1])
                emit_c1_block(3)
                emit_c2_block(4)
                emit_c2_block(5)
                emit_group([8, 9, 10, 11])
                emit_svcs([2, 3, 4, 5])
                emit_c1_block(4)
                emit_mask_bot()
                emit_c2_block(6)
                emit_c2_block(7)
                emit_group([12, 13, 14, 15])
                emit_svcs([6, 7, 8, 9])
                emit_svcs([10, 11, 12, 13])
                emit_svcs([14, 15])
            else:
                emit_deferred_consts()
                for g in range(min(10, cfg.HT_ROWS)):
                    need_ht(g)
                for bi in range(len(cfg.C1_BLOCKS)):
                    emit_c1_block(bi)
                emit_mask_top()
                emit_mask_bot()
                for b in range(n_c2b):
                    emit_c2_block(b)
                for t in range(PAIRS):
                    chains[t] = emit_conv3(t)
                for t in range(PAIRS):
                    emit_svc(t, *chains.pop(t))

    return nc


# ---------------- host side ----------------

_CACHED = None


def _get_graph():
    global _CACHED
    if _CACHED is None:
        _CACHED = build_graph(CFG)
        _CACHED.compile()
    return _CACHED


def make_band_mask(cfg):
    """Static validity mask for band tiles [WH, 10*2*W]."""
    s = np.arange(cfg.WH)[:, None]
    g = (np.arange(10 * 2 * cfg.W)[None, :]) // (2 * cfg.W)
    r = (np.arange(10 * 2 * cfg.W)[None, :] // cfg.W) % 2
    w = np.arange(10 * 2 * cfg.W)[None, :] % cfg.W
    dy = g - r
    dx = s - w
    m = (dy >= 0) & (dy <= 8) & (dx >= 0) & (dx <= 8)
    return m.astype(BF)


def shard_inputs(inputs, cfg):
    """Build per-core input maps from the full problem inputs."""
    cur = np.asarray(inputs["current_frame_low_features"])
    key = np.asarray(inputs["key_frame_low_features"])
    high = np.asarray(inputs["key_frame_high_features"])
    B, Cin, H, W = cur.shape

    w_reduce = np.asarray(inputs["w_reduce"])  # (CO1, Cin, 3, 3)
    w2 = np.asarray(inputs["w2"])  # (CO2, C2, 3, 3)
    w3 = np.asarray(inputs["w3"])  # (81, CO2, 1, 1)
    n_cin_ch = Cin // 128
    n_c2_ch = cfg.C2 // 128
    # w1 host layout [128ci, 9tap, chunk, co]
    w1h = np.ascontiguousarray(
        w_reduce.reshape(cfg.CO1, n_cin_ch, 128, 9).transpose(2, 3, 1, 0)
    ).astype(np.float32)
    w2h = np.ascontiguousarray(
        w2.reshape(cfg.CO2, n_c2_ch, 128, 9).transpose(2, 3, 1, 0)
    ).astype(BF)
    w3h = np.ascontiguousarray(
        w3.reshape(cfg.K81, cfg.CO2 // 128, 128).transpose(2, 1, 0)
    ).astype(BF)
    b1h = np.ascontiguousarray(
        np.asarray(inputs["b_reduce"]).reshape(cfg.CO1 // 128, 128).T
    ).astype(np.float32)
    b2h = np.ascontiguousarray(
        np.asarray(inputs["b2"]).reshape(cfg.CO2 // 128, 128).T
    ).astype(np.float32)
    b3h = np.broadcast_to(
        np.asarray(inputs["b3"]).astype(np.float32)[None, :], (128, cfg.K81)
    ).copy()
    bmask = make_band_mask(cfg)

    in_maps = []
    for core in range(B * cfg.HALVES):
        b, half = core // cfg.HALVES, core % cfg.HALVES
        s = half * cfg.OUT_ROWS
        # low features: rows [s-2, s+OUT_ROWS+2), w padded +-1
        lowpad = np.zeros((2, Cin, cfg.IN_ROWS, cfg.WP), np.float32)
        r0, r1 = s - 2, s + cfg.OUT_ROWS + 2
        cr0, cr1 = max(r0, 0), min(r1, H)
        lowpad[0, :, cr0 - r0 : cr1 - r0, 1 : 1 + W] = cur[b, :, cr0:cr1, :]
        lowpad[1, :, cr0 - r0 : cr1 - r0, 1 : 1 + W] = key[b, :, cr0:cr1, :]
        # high features: rows [s-4, s+OUT_ROWS+4), w padded +-4, transposed
        hp = np.zeros((cfg.HT_ROWS, cfg.WH, cfg.CH), np.float32)
        hr0, hr1 = s - 4, s + cfg.OUT_ROWS + 4
        chr0, chr1 = max(hr0, 0), min(hr1, H)
        hp[chr0 - hr0 : chr1 - hr0, 4 : 4 + W, :] = high[b, :, chr0:chr1, :].transpose(
            1, 2, 0
        )
        hmask = np.zeros((128, 2), np.float32)
        hmask[:, 0] = 0.0 if s == 0 else 1.0
        hmask[:, 1] = 0.0 if s + cfg.OUT_ROWS == H else 1.0
        in_maps.append(
            {
                "cur": lowpad[0],
                "key": lowpad[1],
                "highT": hp.astype(BF),
                "w1": w1h,
                "w2": w2h,
                "w3": w3h,
                "b1": b1h,
                "b2": b2h,
                "b3": b3h,
                "hmask": hmask,
                "bmask": bmask,
            }
        )
    return in_maps


def gather_outputs(results, cfg, H, W):
    out = np.zeros((cfg.B, cfg.CH, H, W), np.float32)
    for core, res in enumerate(results):
        b, half = core // cfg.HALVES, core % cfg.HALVES
        s = half * cfg.OUT_ROWS
        o = np.asarray(res["out"]).reshape(cfg.OUT_ROWS, W, cfg.CH)
        out[b, :, s : s + cfg.OUT_ROWS, :] = o.transpose(2, 0, 1)
    return out


def kernel(**inputs) -> np.ndarray:
    cfg = CFG
    nc = _get_graph()
    in_maps = shard_inputs(inputs, cfg)
    res = run_bass_kernel_spmd(nc, in_maps, core_ids=list(range(8)))
    return gather_outputs(res.results, cfg, cfg.H, cfg.W)



# revision 2
# speedup vs baseline: 1.7897x; 1.7897x over previous
"""AdaptiveFeaturePropagation Trainium2 kernel (8 NeuronCores, SPMD).

Sharding: 8 cores = (batch 4) x (H halves 2); halos replicated host-side, no
collectives. Per core (shard rows [s, s+32)):
  conv1 (3x3, 1024->256, applied to cur & key): bf16 matmuls, contraction
    over Cin in 128-chunks x 9 taps accumulated in PSUM. Inputs arrive
    bf16 in layout [128, rows, chunk*W] so each 7-row block is ONE DMA
    with 9.5KB contiguous runs per partition.
  conv2 (3x3, 512->256) in 8-row blocks (N=512) and conv3 (1x1, 256->81).
  conv3 is computed "swapped" (x3 pixel-block as the stationary operand) so
  kernel logits land pixel-major [128 pix, 81]; softmax = Relu+Exp on ACT with
  accum_out giving the denominator; normalization folded into the SVC drain.
  Spatially-variant 9x9 conv: banded-matrix matmul
    out[pix, c] = sum_g band_g[s,j].T @ highT[g][s, c]
  with the band built by a DMA scatter into a DRAM image at addr 129*p+128*k
  (zero margins), loaded back contiguously, and a static 0/1 mask zeroing
  aliased tap slots.
Output is written pixel-major [2048, 1024] bf16 per core; host transposes
and upcasts.
"""

import sys

sys.path.insert(0, "/opt/trn_rl_repo")

import numpy as np
import ml_dtypes

import concourse.bacc as bacc
import concourse.mybir as mybir
from concourse.bass_types import AP
from concourse.tile import TileContext
from concourse.bass_utils import run_bass_kernel_spmd

F32 = mybir.dt.float32
BF16 = mybir.dt.bfloat16
BF = ml_dtypes.bfloat16
AF = mybir.ActivationFunctionType

# ---------------- configuration ----------------


class Cfg:
    B = 4
    HALVES = 2
    H = 64
    W = 64
    C_IN = 1024  # conv1 input channels
    CO1 = 256  # conv1 output channels (per branch)
    C2 = 512  # conv2 input channels
    CO2 = 256  # conv2 output channels
    K81 = 81
    CH = 1024  # high-feature channels

    OUT_ROWS = 32  # output rows per shard
    # conv1 output rows = OUT_ROWS + 2 (halo +-1), input rows = OUT_ROWS + 4
    C1_BLOCKS = ((0, 7), (7, 7), (14, 7), (21, 7), (28, 6))
    C2_ROWS = 8  # conv2 block rows (N = 512)

    @property
    def X2_ROWS(self):
        return self.OUT_ROWS + 2

    @property
    def IN_ROWS(self):
        return self.OUT_ROWS + 4

    @property
    def HT_ROWS(self):
        return self.OUT_ROWS + 8

    @property
    def PAIRS(self):
        return self.OUT_ROWS // 2

    @property
    def WP(self):
        return self.W + 2

    @property
    def WH(self):
        return self.W + 8

    # band image geometry: addr = 129*p + 128*k, p in [0,128), k in [0,81)
    XPW = 129  # p stride
    XKW = 128  # k stride

    @property
    def XR(self):  # r offset: 129*64
        return self.XPW * self.W

    @property
    def XSZ(self):  # per-pair region size (max read addr + pad)
        return 26752


CFG = Cfg()

# ---------------- graph builder ----------------


def build_graph(cfg):
    nc = bacc.Bacc(None, target_bir_lowering=False)
    W = cfg.W
    n_cin_ch = cfg.C_IN // 128
    n_c2_ch = cfg.C2 // 128
    n_co1_h = cfg.CO1 // 128
    n_co2_h = cfg.CO2 // 128
    n_cc = cfg.CH // 512  # SVC c-chunks
    PAIRS = cfg.PAIRS
    CWP = n_cin_ch * cfg.WP  # 528: chunk*W row pitch

    cur_e = nc.declare_dram_parameter(
        "cur", [128, cfg.IN_ROWS, CWP], BF16, isOutput=False
    )
    key_e = nc.declare_dram_parameter(
        "key", [128, cfg.IN_ROWS, CWP], BF16, isOutput=False
    )
    high_e = nc.declare_dram_parameter(
        "highT", [cfg.HT_ROWS, cfg.WH, cfg.CH], BF16, isOutput=False
    )
    w1_e = nc.declare_dram_parameter(
        "w1", [128, 9, n_cin_ch, cfg.CO1], BF16, isOutput=False
    )
    w2_e = nc.declare_dram_parameter(
        "w2", [128, 9, n_c2_ch, cfg.CO2], BF16, isOutput=False
    )
    w3_e = nc.declare_dram_parameter(
        "w3", [128, cfg.CO2 // 128, cfg.K81], BF16, isOutput=False
    )
    b1_e = nc.declare_dram_parameter("b1", [128, n_co1_h], F32, isOutput=False)
    b2_e = nc.declare_dram_parameter("b2", [128, n_co2_h], F32, isOutput=False)
    b3_e = nc.declare_dram_parameter("b3", [128, cfg.K81], F32, isOutput=False)
    hmask_e = nc.declare_dram_parameter("hmask", [128, 2], F32, isOutput=False)
    bmask_e = nc.declare_dram_parameter(
        "bmask", [cfg.WH, 10 * 2 * W], BF16, isOutput=False
    )
    out_e = nc.declare_dram_parameter(
        "out", [cfg.OUT_ROWS * W, cfg.CH], BF16, isOutput=True
    )

    ximg = nc.dram_tensor("ximg", [PAIRS * cfg.XSZ], BF16)

    with TileContext(nc) as tc:
        with (
            tc.tile_pool(name="const", bufs=1) as cpool,
            tc.tile_pool(name="feat", bufs=1) as fpool,
            tc.tile_pool(name="c1in", bufs=4) as inpool,
            tc.tile_pool(name="ht", bufs=12) as htpool,
            tc.tile_pool(name="band", bufs=8) as bandpool,
            tc.tile_pool(name="small", bufs=8) as spool,
            tc.tile_pool(name="ob", bufs=3) as obpool,
            tc.tile_pool(name="ps", bufs=8, space="PSUM") as pspool,
        ):
            # ---- persistent constants ----
            w1sb = cpool.tile([128, 9 * n_cin_ch * cfg.CO1], BF16)
            nc.scalar.dma_start(out=w1sb[:], in_=w1_e[:, :, :, :])
            b1sb = cpool.tile([128, n_co1_h], F32)
            nc.scalar.dma_start(out=b1sb[:], in_=b1_e[:, :])
            hmsb = cpool.tile([128, 2], F32)
            nc.scalar.dma_start(out=hmsb[:], in_=hmask_e[:, :])
            w2sb = cpool.tile([128, 9 * n_c2_ch * cfg.CO2], BF16)
            w3sb = cpool.tile([128, (cfg.CO2 // 128) * cfg.K81], BF16)
            b2sb = cpool.tile([128, n_co2_h], F32)
            b3sb = cpool.tile([128, cfg.K81], F32)
            bmsb = cpool.tile([cfg.WH, 10 * 2 * W], BF16)
            zt = cpool.tile([128, cfg.XSZ // 128], BF16)

            def emit_deferred_consts():
                nc.scalar.dma_start(out=w2sb[:], in_=w2_e[:, :, :, :])
                nc.scalar.dma_start(out=w3sb[:], in_=w3_e[:, :, :])
                nc.scalar.dma_start(out=b2sb[:], in_=b2_e[:, :])
                nc.scalar.dma_start(out=b3sb[:], in_=b3_e[:, :])
                nc.scalar.dma_start(out=bmsb[:], in_=bmask_e[:, :])
                nc.vector.memset(zt[:], 0.0)
                for t in range(PAIRS):
                    dz = AP(
                        ximg,
                        t * cfg.XSZ,
                        [[cfg.XSZ // 128, 128], [1, cfg.XSZ // 128]],
                    )
                    nc.scalar.dma_start(out=dz, in_=zt[:])

            # x2 (conv1 out, conv2 in), bf16, padded cols; x3 (conv2 out)
            x2c = []
            for i in range(2 * n_co1_h):
                t_ = fpool.tile([128, cfg.X2_ROWS * cfg.WP], BF16, tag=f"x2_{i}")
                nc.vector.memset(t_[:], 0.0)
                x2c.append(t_)
            x3c = []
            for i in range(n_co2_h):
                t_ = fpool.tile([128, cfg.OUT_ROWS * W], BF16, tag=f"x3_{i}")
                x3c.append(t_)

            # highT ring
            ht = {}

            def need_ht(g):
                if g not in ht:
                    h_ = htpool.tile([cfg.WH, cfg.CH], BF16, tag="ht")
                    nc.scalar.dma_start(out=h_[:], in_=high_e[g, :, :])
                    ht[g] = h_
                return ht[g]

            # ---- conv1 (cur, key) -> x2 ----
            def emit_c1_block(bi):
                o0, nout = cfg.C1_BLOCKS[bi]
                nin = nout + 2
                for ii, inp_e in enumerate((cur_e, key_e)):
                    it = inpool.tile([128, nin * CWP], BF16, tag="c1in")
                    nc.scalar.dma_start(out=it[:], in_=inp_e[:, o0 : o0 + nin, :])
                    itv = it[:, :].rearrange(
                        "p (r c w) -> p r c w", c=n_cin_ch, w=cfg.WP
                    )
                    pss = [
                        pspool.tile(
                            [128, nout * W], F32, tag="ps", name=f"ps1_{o0}_{ii}_{h_}"
                        )
                        for h_ in range(n_co1_h)
                    ]
                    for ch in range(n_cin_ch):
                        for tap in range(9):
                            dy, dx = tap // 3, tap % 3
                            rhs = itv[:, dy : dy + nout, ch, dx : dx + W]
                            for hf in range(n_co1_h):
                                lhsT = w1sb[
                                    :,
                                    (tap * n_cin_ch + ch) * cfg.CO1
                                    + 128 * hf : (tap * n_cin_ch + ch) * cfg.CO1
                                    + 128 * (hf + 1),
                                ]
                                nc.tensor.matmul(
                                    pss[hf][:, :],
                                    lhsT,
                                    rhs,
                                    start=(ch == 0 and tap == 0),
                                    stop=(ch == n_cin_ch - 1 and tap == 8),
                                )
                    for hf in range(n_co1_h):
                        dst = x2c[ii * n_co1_h + hf][:, :].rearrange(
                            "p (r w) -> p r w", w=cfg.WP
                        )[:, o0 : o0 + nout, 1 : 1 + W]
                        nc.scalar.activation(
                            dst,
                            pss[hf][:, :],
                            AF.Relu,
                            bias=b1sb[:, hf : hf + 1],
                        )

            # halo row masking (rows 0 and X2_ROWS-1 of x2)
            lr = cfg.X2_ROWS - 1

            def emit_mask_top():
                for i in range(2 * n_co1_h):
                    nc.vector.tensor_scalar_mul(
                        x2c[i][:, 0 : cfg.WP], x2c[i][:, 0 : cfg.WP], hmsb[:, 0:1]
                    )

            def emit_mask_bot():
                for i in range(2 * n_co1_h):
                    nc.vector.tensor_scalar_mul(
                        x2c[i][:, lr * cfg.WP : (lr + 1) * cfg.WP],
                        x2c[i][:, lr * cfg.WP : (lr + 1) * cfg.WP],
                        hmsb[:, 1:2],
                    )

            # ---- conv2 -> x3 (8-row blocks, N = 512) ----
            def emit_c2_block(b):
                nr = cfg.C2_ROWS
                pss = [
                    pspool.tile([128, nr * W], F32, tag="ps", name=f"ps2_{b}_{h_}")
                    for h_ in range(n_co2_h)
                ]
                for ch in range(n_c2_ch):
                    x2v = x2c[ch][:, :].rearrange("p (r w) -> p r w", w=cfg.WP)
                    for tap in range(9):
                        dy, dx = tap // 3, tap % 3
                        rhs = x2v[:, nr * b + dy : nr * b + dy + nr, dx : dx + W]
                        for hf in range(n_co2_h):
                            lhsT = w2sb[
                                :,
                                (tap * n_c2_ch + ch) * cfg.CO2
                                + 128 * hf : (tap * n_c2_ch + ch) * cfg.CO2
                                + 128 * (hf + 1),
                            ]
                            nc.tensor.matmul(
                                pss[hf][:, :],
                                lhsT,
                                rhs,
                                start=(ch == 0 and tap == 0),
                                stop=(ch == n_c2_ch - 1 and tap == 8),
                            )
                for hf in range(n_co2_h):
                    nc.scalar.activation(
                        x3c[hf][:, b * nr * W : (b + 1) * nr * W],
                        pss[hf][:, :],
                        AF.Relu,
                        bias=b2sb[:, hf : hf + 1],
                    )

            # ---- per row-pair: conv3 + softmax + band + SVC ----
            def emit_conv3(t):
                ps3 = pspool.tile([128, cfg.K81], F32, tag="ps")
                for ch in range(cfg.CO2 // 128):
                    nc.tensor.matmul(
                        ps3[:, :],
                        x3c[ch][:, t * 128 : (t + 1) * 128],
                        w3sb[:, ch * cfg.K81 : (ch + 1) * cfg.K81],
                        start=(ch == 0),
                        stop=(ch == cfg.CO2 // 128 - 1),
                    )
                t81 = spool.tile([128, cfg.K81], F32, tag="t81")
                nc.vector.tensor_add(t81[:], ps3[:, :], b3sb[:])
                nc.scalar.activation(t81[:], t81[:], AF.Relu)
                kt = spool.tile([128, cfg.K81], BF16, tag="kt")
                dsum = spool.tile([128, 1], F32, tag="dsum")
                nc.scalar.activation(kt[:], t81[:], AF.Exp, accum_out=dsum[:])
                rd = spool.tile([128, 1], F32, tag="rd")
                nc.vector.reciprocal(rd[:], dsum[:])
                # scatter into band image (SWDGE: separate queue from sync DMAs)
                for r in range(2):
                    dstap = AP(
                        ximg,
                        t * cfg.XSZ + cfg.XR * r,
                        [[cfg.XPW, W], [cfg.XKW, cfg.K81]],
                    )
                    nc.gpsimd.dma_start(out=dstap, in_=kt[W * r : W * (r + 1), :])
                # band load + mask
                band = bandpool.tile([cfg.WH, 10 * 2 * W], BF16, tag="band")
                for r in range(2):
                    srcap = AP(
                        ximg,
                        t * cfg.XSZ + (cfg.XR - 9 * cfg.XKW) * r,
                        [[cfg.XKW, cfg.WH], [cfg.XKW * 9, 10], [1, W]],
                    )
                    dstap = band[:, :].rearrange(
                        "s (g rr w) -> s g rr w", g=10, rr=2
                    )[:, :, r, :]
                    nc.sync.dma_start(out=dstap, in_=srcap)
                nc.vector.tensor_mul(band[:], band[:], bmsb[:])
                return band, rd

            def emit_svc(t, band, rd):
                for g in range(2 * t, 2 * t + 10):
                    need_ht(g)
                for cc in range(n_cc):
                    pv = pspool.tile([128, 512], F32, tag="ps")
                    for gi in range(10):
                        nc.tensor.matmul(
                            pv[:, :],
                            band[:, 128 * gi : 128 * (gi + 1)],
                            ht[2 * t + gi][:, 512 * cc : 512 * (cc + 1)],
                            start=(gi == 0),
                            stop=(gi == 9),
                        )
                    ob = obpool.tile([128, 512], BF16, tag="ob")
                    nc.scalar.activation(
                        ob[:], pv[:, :], AF.Copy, scale=rd[:, 0:1]
                    )
                    nc.sync.dma_start(
                        out=out_e[t * 128 : (t + 1) * 128, 512 * cc : 512 * (cc + 1)],
                        in_=ob[:],
                    )

            # interleaved schedule: spread band scatters across conv compute
            chains = {}

            def emit_group(ts):
                for t in ts:
                    chains[t] = emit_conv3(t)

            def emit_svcs(ts):
                for t in ts:
                    emit_svc(t, *chains.pop(t))

            n_c2b = cfg.OUT_ROWS // cfg.C2_ROWS
            if n_c2b == 4 and len(cfg.C1_BLOCKS) == 5:
                emit_c1_block(0)
                emit_mask_top()
                emit_deferred_consts()
                for g in range(10):
                    need_ht(g)
                emit_c1_block(1)
                emit_c2_block(0)
                emit_group([0, 1, 2, 3])
                emit_c1_block(2)
                emit_c2_block(1)
                emit_group([4, 5, 6, 7])
                emit_svcs([0, 1])
                emit_c1_block(3)
                emit_c2_block(2)
                emit_group([8, 9, 10, 11])
                emit_svcs([2, 3, 4, 5])
                emit_c1_block(4)
                emit_mask_bot()
                emit_c2_block(3)
                emit_group([12, 13, 14, 15])
                emit_svcs([6, 7, 8, 9])
                emit_svcs([10, 11, 12, 13])
                emit_svcs([14, 15])
            else:
                emit_deferred_consts()
                for g in range(min(10, cfg.HT_ROWS)):
                    need_ht(g)
                for bi in range(len(cfg.C1_BLOCKS)):
                    emit_c1_block(bi)
                emit_mask_top()
                emit_mask_bot()
                for b in range(n_c2b):
                    emit_c2_block(b)
                for t in range(PAIRS):
                    chains[t] = emit_conv3(t)
                for t in range(PAIRS):
                    emit_svc(t, *chains.pop(t))

    return nc


# ---------------- host side ----------------

_CACHED = None


def _get_graph():
    global _CACHED
    if _CACHED is None:
        _CACHED = build_graph(CFG)
        _CACHED.compile()
    return _CACHED


def make_band_mask(cfg):
    """Static validity mask for band tiles [WH, 10*2*W]."""
    s = np.arange(cfg.WH)[:, None]
    g = (np.arange(10 * 2 * cfg.W)[None, :]) // (2 * cfg.W)
    r = (np.arange(10 * 2 * cfg.W)[None, :] // cfg.W) % 2
    w = np.arange(10 * 2 * cfg.W)[None, :] % cfg.W
    dy = g - r
    dx = s - w
    m = (dy >= 0) & (dy <= 8) & (dx >= 0) & (dx <= 8)
    return m.astype(BF)


def shard_inputs(inputs, cfg):
    """Build per-core input maps from the full problem inputs."""
    cur = np.asarray(inputs["current_frame_low_features"])
    key = np.asarray(inputs["key_frame_low_features"])
    high = np.asarray(inputs["key_frame_high_features"])
    B, Cin, H, W = cur.shape

    w_reduce = np.asarray(inputs["w_reduce"])  # (CO1, Cin, 3, 3)
    w2 = np.asarray(inputs["w2"])  # (CO2, C2, 3, 3)
    w3 = np.asarray(inputs["w3"])  # (81, CO2, 1, 1)
    n_cin_ch = Cin // 128
    n_c2_ch = cfg.C2 // 128
    # w1 host layout [128ci, 9tap, chunk, co]
    w1h = np.ascontiguousarray(
        w_reduce.reshape(cfg.CO1, n_cin_ch, 128, 9).transpose(2, 3, 1, 0)
    ).astype(BF)
    w2h = np.ascontiguousarray(
        w2.reshape(cfg.CO2, n_c2_ch, 128, 9).transpose(2, 3, 1, 0)
    ).astype(BF)
    w3h = np.ascontiguousarray(
        w3.reshape(cfg.K81, cfg.CO2 // 128, 128).transpose(2, 1, 0)
    ).astype(BF)
    b1h = np.ascontiguousarray(
        np.asarray(inputs["b_reduce"]).reshape(cfg.CO1 // 128, 128).T
    ).astype(np.float32)
    b2h = np.ascontiguousarray(
        np.asarray(inputs["b2"]).reshape(cfg.CO2 // 128, 128).T
    ).astype(np.float32)
    b3h = np.broadcast_to(
        np.asarray(inputs["b3"]).astype(np.float32)[None, :], (128, cfg.K81)
    ).copy()
    bmask = make_band_mask(cfg)

    in_maps = []
    for core in range(B * cfg.HALVES):
        b, half = core // cfg.HALVES, core % cfg.HALVES
        s = half * cfg.OUT_ROWS
        # low features: rows [s-2, s+OUT_ROWS+2), w padded +-1, bf16,
        # layout [128, IN_ROWS, chunk*WP]
        lowpad = np.zeros((2, Cin, cfg.IN_ROWS, cfg.WP), np.float32)
        r0, r1 = s - 2, s + cfg.OUT_ROWS + 2
        cr0, cr1 = max(r0, 0), min(r1, H)
        lowpad[0, :, cr0 - r0 : cr1 - r0, 1 : 1 + W] = cur[b, :, cr0:cr1, :]
        lowpad[1, :, cr0 - r0 : cr1 - r0, 1 : 1 + W] = key[b, :, cr0:cr1, :]
        lowT = np.ascontiguousarray(
            lowpad.reshape(2, n_cin_ch, 128, cfg.IN_ROWS, cfg.WP).transpose(
                0, 2, 3, 1, 4
            )
        ).reshape(2, 128, cfg.IN_ROWS, n_cin_ch * cfg.WP).astype(BF)
        # high features: rows [s-4, s+OUT_ROWS+4), w padded +-4, transposed
        hp = np.zeros((cfg.HT_ROWS, cfg.WH, cfg.CH), np.float32)
        hr0, hr1 = s - 4, s + cfg.OUT_ROWS + 4
        chr0, chr1 = max(hr0, 0), min(hr1, H)
        hp[chr0 - hr0 : chr1 - hr0, 4 : 4 + W, :] = high[b, :, chr0:chr1, :].transpose(
            1, 2, 0
        )
        hmask = np.zeros((128, 2), np.float32)
        hmask[:, 0] = 0.0 if s == 0 else 1.0
        hmask[:, 1] = 0.0 if s + cfg.OUT_ROWS == H else 1.0
        in_maps.append(
            {
                "cur": lowT[0],
                "key": lowT[1],
                "highT": hp.astype(BF),
                "w1": w1h,
                "w2": w2h,
                "w3": w3h,
                "b1": b1h,
                "b2": b2h,
                "b3": b3h,
                "hmask": hmask,
                "bmask": bmask,
            }
        )
    return in_maps


def gather_outputs(results, cfg, H, W):
    out = np.zeros((cfg.B, cfg.CH, H, W), np.float32)
    for core, res in enumerate(results):
        b, half = core // cfg.HALVES, core % cfg.HALVES
        s = half * cfg.OUT_ROWS
        o = np.asarray(res["out"]).astype(np.float32).reshape(
            cfg.OUT_ROWS, W, cfg.CH
        )
        out[b, :, s : s + cfg.OUT_ROWS, :] = o.transpose(2, 0, 1)
    return out


def kernel(**inputs) -> np.ndarray:
    cfg = CFG
    nc = _get_graph()
    in_maps = shard_inputs(inputs, cfg)
    res = run_bass_kernel_spmd(nc, in_maps, core_ids=list(range(8)))
    return gather_outputs(res.results, cfg, cfg.H, cfg.W)
